# revision 1
# baseline (speedup 1.0000x reference)
"""GRU cell (single timestep) on 8 TRN2 NeuronCores, data-parallel over batch.

Contract: kernel(**inputs) takes FULL numpy inputs (as produced by the
problem's setup_inputs()) and returns the FULL (16384, 1024) float32 output.

Strategy:
  - Shard batch (16384) across 8 cores -> 2048 rows/core. Replicate weights.
  - Host-side packing puts every tensor in feature-major ("transposed world")
    layout so the TensorEngine contraction dim is the partition dim and no
    on-chip transposes are needed:
      xT   [128, 4, 2048]  bf16   [p, k, b] = x[b, 128k+p]
      hT   [128, 8, 2048]  f32    [p, k, b] = hidden[b, 128k+p]
      W**  [128, K, 1024]  bf16   [p, k, o] = W[128k+p, o]   (K=4 for x-side, 8 for h-side)
      bias [128, 24]       f32    [p, 8g+m] = b_g[128m+p]
      outT [128, 8, 2048]  f32    [p, m, b] = out[b, 128m+p]
  - All matmuls in bf16 (PE runs 4x slower on fp32), fp32 PSUM accumulation,
    all elementwise/activations in fp32 with the fp32 hidden state.
"""

import sys

if "/opt/trn_rl_repo" not in sys.path:
    sys.path.insert(0, "/opt/trn_rl_repo")

import numpy as np
import ml_dtypes

import concourse.bass as bass
import concourse.tile as tile
from concourse import bacc, mybir
from concourse.bass_utils import run_bass_kernel_spmd

P = 128
NCORES = 8
BATCH = 16384
NB = BATCH // NCORES          # 2048 rows per core
IN = 512
HID = 1024
KX = IN // P                  # 4
KH = HID // P                 # 8
M = HID // P                  # 8 output-feature chunks
BLK = 512                     # batch columns per block
NBLK = NB // BLK              # 4

F32 = mybir.dt.float32
BF16 = mybir.dt.bfloat16

_CACHE = {}


def _build():
    nc = bacc.Bacc("TRN2", target_bir_lowering=False, debug=False, num_devices=NCORES)

    xT = nc.dram_tensor("xT", [P, KX, NB], BF16, kind="ExternalInput").ap()
    hT = nc.dram_tensor("hT", [P, KH, NB], F32, kind="ExternalInput").ap()
    wxr = nc.dram_tensor("wxr", [P, KX, HID], BF16, kind="ExternalInput").ap()
    wxz = nc.dram_tensor("wxz", [P, KX, HID], BF16, kind="ExternalInput").ap()
    wxh = nc.dram_tensor("wxh", [P, KX, HID], BF16, kind="ExternalInput").ap()
    whr = nc.dram_tensor("whr", [P, KH, HID], BF16, kind="ExternalInput").ap()
    whz = nc.dram_tensor("whz", [P, KH, HID], BF16, kind="ExternalInput").ap()
    whh = nc.dram_tensor("whh", [P, KH, HID], BF16, kind="ExternalInput").ap()
    bias = nc.dram_tensor("bias", [P, 24], F32, kind="ExternalInput").ap()
    outT = nc.dram_tensor("outT", [P, M, NB], F32, kind="ExternalOutput").ap()

    with tile.TileContext(nc) as tc:
        with (
            tc.tile_pool(name="wpool", bufs=1) as wpool,
            tc.tile_pool(name="xpool", bufs=2) as xpool,
            tc.tile_pool(name="hpool", bufs=2) as hpool,
            tc.tile_pool(name="hbbpool", bufs=1) as hbbpool,
            tc.tile_pool(name="rpool", bufs=2) as rpool,
            tc.tile_pool(name="rhpool", bufs=1) as rhpool,
            tc.tile_pool(name="zpool", bufs=1) as zpool,
            tc.tile_pool(name="hcpool", bufs=2) as hcpool,
            tc.tile_pool(name="opool", bufs=3) as opool,
            tc.tile_pool(name="psum", bufs=6, space=bass.MemorySpace.PSUM) as psum,
        ):
            # resident weights + bias
            wxr_s = wpool.tile([P, KX, HID], BF16)
            wxz_s = wpool.tile([P, KX, HID], BF16)
            wxh_s = wpool.tile([P, KX, HID], BF16)
            whr_s = wpool.tile([P, KH, HID], BF16)
            whz_s = wpool.tile([P, KH, HID], BF16)
            whh_s = wpool.tile([P, KH, HID], BF16)
            b_s = wpool.tile([P, 24], F32)
            nc.sync.dma_start(wxr_s[:], wxr[:])
            nc.sync.dma_start(wxz_s[:], wxz[:])
            nc.sync.dma_start(wxh_s[:], wxh[:])
            nc.sync.dma_start(whr_s[:], whr[:])
            nc.sync.dma_start(whz_s[:], whz[:])
            nc.sync.dma_start(whh_s[:], whh[:])
            nc.sync.dma_start(b_s[:], bias[:])

            for blk in range(NBLK):
                sl = bass.ts(blk, BLK)
                xb = xpool.tile([P, KX, BLK], BF16, tag="xb")
                nc.sync.dma_start(xb[:], xT[:, :, sl])
                hb = hpool.tile([P, KH, BLK], F32, tag="hb")
                nc.sync.dma_start(hb[:], hT[:, :, sl])
                hbb = hbbpool.tile([P, KH, BLK], BF16, tag="hbb")
                nc.vector.tensor_copy(hbb[:], hb[:])

                rh = rhpool.tile([P, KH, BLK], BF16, tag="rh")

                # ---- R phase: r = sigmoid(x@Wxr + bxr + h@Whr); rh = r*h
                for m in range(M):
                    ps = psum.tile([P, BLK], F32, tag="ps")
                    mo = bass.ts(m, P)
                    for k in range(KX):
                        nc.tensor.matmul(
                            ps[:], wxr_s[:, k, mo], xb[:, k, :],
                            start=(k == 0), stop=False,
                        )
                    for k in range(KH):
                        nc.tensor.matmul(
                            ps[:], whr_s[:, k, mo], hbb[:, k, :],
                            start=False, stop=(k == KH - 1),
                        )
                    rt = rpool.tile([P, BLK], F32, tag="rt")
                    nc.scalar.activation(
                        rt[:], ps[:], mybir.ActivationFunctionType.Sigmoid,
                        bias=b_s[:, m : m + 1],
                    )
                    nc.vector.tensor_mul(rh[:, m, :], rt[:], hb[:, m, :])

                # ---- Z phase: z = sigmoid(x@Wxz + bxz + h@Whz)
                zf = zpool.tile([P, M, BLK], F32, tag="zf")
                for m in range(M):
                    ps = psum.tile([P, BLK], F32, tag="ps")
                    mo = bass.ts(m, P)
                    for k in range(KX):
                        nc.tensor.matmul(
                            ps[:], wxz_s[:, k, mo], xb[:, k, :],
                            start=(k == 0), stop=False,
                        )
                    for k in range(KH):
                        nc.tensor.matmul(
                            ps[:], whz_s[:, k, mo], hbb[:, k, :],
                            start=False, stop=(k == KH - 1),
                        )
                    nc.scalar.activation(
                        zf[:, m, :], ps[:], mybir.ActivationFunctionType.Sigmoid,
                        bias=b_s[:, 8 + m : 9 + m],
                    )

                # ---- HC phase: hc = tanh(x@Wxh + bxh + rh@Whh); out = hc + z*(h-hc)
                for m in range(M):
                    ps = psum.tile([P, BLK], F32, tag="ps")
                    mo = bass.ts(m, P)
                    for k in range(KX):
                        nc.tensor.matmul(
                            ps[:], wxh_s[:, k, mo], xb[:, k, :],
                            start=(k == 0), stop=False,
                        )
                    for k in range(KH):
                        nc.tensor.matmul(
                            ps[:], whh_s[:, k, mo], rh[:, k, :],
                            start=False, stop=(k == KH - 1),
                        )
                    hct = hcpool.tile([P, BLK], F32, tag="hct")
                    nc.scalar.activation(
                        hct[:], ps[:], mybir.ActivationFunctionType.Tanh,
                        bias=b_s[:, 16 + m : 17 + m],
                    )
                    ot = opool.tile([P, BLK], F32, tag="ot")
                    nc.vector.tensor_sub(ot[:], hb[:, m, :], hct[:])
                    nc.vector.tensor_mul(ot[:], ot[:], zf[:, m, :])
                    nc.vector.tensor_add(ot[:], ot[:], hct[:])
                    nc.sync.dma_start(outT[:, m, sl], ot[:])

    nc.compile()
    return nc


def _pack_feature_major(a: np.ndarray, nchunks: int, dtype) -> np.ndarray:
    # [rows, cols] -> [128, nchunks, cols] with [p, k, c] = a[128k+p, c]
    rows, cols = a.shape
    assert rows == nchunks * P
    return np.ascontiguousarray(
        a.reshape(nchunks, P, cols).transpose(1, 0, 2)
    ).astype(dtype)


def kernel(x, hidden, Wxr, bxr, Whr, Wxz, bxz, Whz, Wxh, bxh, Whh):
    if "nc" not in _CACHE:
        _CACHE["nc"] = _build()
    nc = _CACHE["nc"]

    bf = ml_dtypes.bfloat16
    wxr_p = _pack_feature_major(np.asarray(Wxr, np.float32), KX, bf)
    wxz_p = _pack_feature_major(np.asarray(Wxz, np.float32), KX, bf)
    wxh_p = _pack_feature_major(np.asarray(Wxh, np.float32), KX, bf)
    whr_p = _pack_feature_major(np.asarray(Whr, np.float32), KH, bf)
    whz_p = _pack_feature_major(np.asarray(Whz, np.float32), KH, bf)
    whh_p = _pack_feature_major(np.asarray(Whh, np.float32), KH, bf)
    bias_p = np.ascontiguousarray(
        np.concatenate(
            [
                np.asarray(b, np.float32).reshape(M, P).T
                for b in (bxr, bxz, bxh)
            ],
            axis=1,
        )
    )  # [128, 24]

    x = np.asarray(x, np.float32)
    hidden = np.asarray(hidden, np.float32)

    in_maps = []
    for c in range(NCORES):
        rows = slice(c * NB, (c + 1) * NB)
        xT_p = _pack_feature_major(x[rows].T, KX, bf)        # [128, 4, 2048]
        hT_p = _pack_feature_major(hidden[rows].T, KH, np.float32)
        in_maps.append(
            {
                "xT": xT_p,
                "hT": hT_p,
                "wxr": wxr_p,
                "wxz": wxz_p,
                "wxh": wxh_p,
                "whr": whr_p,
                "whz": whz_p,
                "whh": whh_p,
                "bias": bias_p,
            }
        )

    res = run_bass_kernel_spmd(nc, in_maps, core_ids=list(range(NCORES)))

    out = np.empty((BATCH, HID), np.float32)
    for c in range(NCORES):
        oT = res.results[c]["outT"]  # [128, 8, 2048]
        out[c * NB : (c + 1) * NB] = oT.transpose(1, 0, 2).reshape(HID, NB).T
    return out



# revision 3
# speedup vs baseline: 1.3552x; 1.3552x over previous
"""GRU cell (single timestep) on 8 TRN2 NeuronCores, data-parallel over batch.

Contract: kernel(**inputs) takes FULL numpy inputs (as produced by the
problem's setup_inputs()) and returns the FULL (16384, 1024) float32 output.

Strategy:
  - Shard batch (16384) across 8 cores -> 2048 rows/core. Replicate weights.
  - Host-side packing puts every tensor in feature-major layout so the
    TensorEngine contraction dim is the partition dim:
      xT   [128, 4, 2048]  bf16   [p, k, b] = x[b, 128k+p]
      hT   [128, 8, 2048]  bf16   [p, k, b] = hidden[b, 128k+p]
      Wx*  [128, 4, 1024]  bf16   [p, k, o] = 1024*W[128k+p, o]
      Wh*  [128, 8, 1024]  fp8e4  [p, k, o] = e4m3(1024*W[128k+p, o])
      bias [128, 24]       f32    [p, 8g+m] = b_g[128m+p]
      outT [128, 8, 2048]  bf16   [p, m, b] = out[b, 128m+p]
  - x-side matmuls in bf16; h-side matmuls in fp8 e4m3 with
    perf_mode=DoubleRow (2 k-tiles contracted per MM -> ~1.5-1.8x PE rate).
    Both weight sets are pre-scaled by 1024 (exact pow2 shift for bf16;
    lifts fp8 weights out of the subnormal range); the activation applies
    scale=1/1024 before bias+nonlinearity, so numerics are unchanged.
  - fp32 PSUM accumulation; activations in fp32 on the Scalar engine;
    elementwise combine split across Vector and GpSimd engines.
"""

import sys

if "/opt/trn_rl_repo" not in sys.path:
    sys.path.insert(0, "/opt/trn_rl_repo")

import numpy as np
import ml_dtypes

import concourse.bass as bass
import concourse.tile as tile
from concourse import bacc, mybir
from concourse.bass_utils import run_bass_kernel_spmd

P = 128
NCORES = 8
BATCH = 16384
NB = BATCH // NCORES          # 2048 rows per core
IN = 512
HID = 1024
KX = IN // P                  # 4
KH = HID // P                 # 8
M = HID // P                  # 8 output-feature chunks
BLK = 512                     # batch columns per block
NBLK = NB // BLK              # 4
WSCALE = 1024.0               # pow2 pre-scale on all weights

F32 = mybir.dt.float32
BF16 = mybir.dt.bfloat16
FP8 = mybir.dt.float8e4

SIG = mybir.ActivationFunctionType.Sigmoid
TANH = mybir.ActivationFunctionType.Tanh
DR = mybir.MatmulPerfMode.DoubleRow

_CACHE = {}


def _build():
    nc = bacc.Bacc("TRN2", target_bir_lowering=False, debug=False, num_devices=NCORES)

    xT = nc.dram_tensor("xT", [P, KX, NB], BF16, kind="ExternalInput").ap()
    hT = nc.dram_tensor("hT", [P, KH, NB], BF16, kind="ExternalInput").ap()
    wxr = nc.dram_tensor("wxr", [P, KX, HID], BF16, kind="ExternalInput").ap()
    wxz = nc.dram_tensor("wxz", [P, KX, HID], BF16, kind="ExternalInput").ap()
    wxh = nc.dram_tensor("wxh", [P, KX, HID], BF16, kind="ExternalInput").ap()
    whr = nc.dram_tensor("whr", [P, KH, HID], FP8, kind="ExternalInput").ap()
    whz = nc.dram_tensor("whz", [P, KH, HID], FP8, kind="ExternalInput").ap()
    whh = nc.dram_tensor("whh", [P, KH, HID], FP8, kind="ExternalInput").ap()
    bias = nc.dram_tensor("bias", [P, 24], F32, kind="ExternalInput").ap()
    outT = nc.dram_tensor("outT", [P, M, NB], BF16, kind="ExternalOutput").ap()

    inv_s = 1.0 / WSCALE

    with tile.TileContext(nc) as tc:
        with (
            tc.tile_pool(name="wpool", bufs=1) as wpool,
            tc.tile_pool(name="xpool", bufs=2) as xpool,
            tc.tile_pool(name="hpool", bufs=2) as hpool,
            tc.tile_pool(name="h8pool", bufs=2) as h8pool,
            tc.tile_pool(name="rpool", bufs=2) as rpool,
            tc.tile_pool(name="rhpool", bufs=2) as rhpool,
            tc.tile_pool(name="zpool", bufs=2) as zpool,
            tc.tile_pool(name="hcpool", bufs=2) as hcpool,
            tc.tile_pool(name="opool", bufs=4) as opool,
            tc.tile_pool(name="psum", bufs=8, space=bass.MemorySpace.PSUM) as psum,
        ):
            # resident weights + bias; DMA priority order: what the first
            # matmuls need comes first so the PE can start ~35us earlier.
            b_s = wpool.tile([P, 24], F32)
            wxr_s = wpool.tile([P, KX, HID], BF16)
            whr_s = wpool.tile([P, KH, HID], FP8)
            wxz_s = wpool.tile([P, KX, HID], BF16)
            whz_s = wpool.tile([P, KH, HID], FP8)
            wxh_s = wpool.tile([P, KX, HID], BF16)
            whh_s = wpool.tile([P, KH, HID], FP8)

            xb = [None] * NBLK
            hb = [None] * NBLK
            h8 = [None] * NBLK

            def fetch_block(blk):
                sl = bass.ts(blk, BLK)
                xb[blk] = xpool.tile([P, KX, BLK], BF16, tag="xb", name="xb")
                nc.sync.dma_start(xb[blk][:], xT[:, :, sl])
                hb[blk] = hpool.tile([P, KH, BLK], BF16, tag="hb", name="hb")
                nc.sync.dma_start(hb[blk][:], hT[:, :, sl])
                h8[blk] = h8pool.tile([P, KH, BLK], FP8, tag="h8", name="h8")
                eng = nc.vector if blk % 2 == 0 else nc.gpsimd
                eng.tensor_copy(h8[blk][:], hb[blk][:])

            nc.sync.dma_start(b_s[:], bias[:])
            fetch_block(0)
            nc.sync.dma_start(wxr_s[:], wxr[:])
            nc.sync.dma_start(whr_s[:], whr[:])
            nc.sync.dma_start(wxz_s[:], wxz[:])
            nc.sync.dma_start(whz_s[:], whz[:])
            nc.sync.dma_start(wxh_s[:], wxh[:])
            nc.sync.dma_start(whh_s[:], whh[:])

            def mm_group(ps, wx_s, wh_s, m, xbt, rhs8, first_x):
                """One PSUM accumulation group: 4 bf16 x-side MMs + 4 fp8
                DoubleRow h-side MMs -> ps = 1024*(x@Wx + rhs@Wh)[mo]."""
                mo = bass.ts(m, P)
                for k in range(KX):
                    nc.tensor.matmul(
                        ps[:], wx_s[:, k, mo], xbt[:, k, :],
                        start=(first_x and k == 0), stop=False,
                    )
                for j in range(KH // 2):
                    nc.tensor.matmul(
                        ps[:], wh_s[:, 2 * j : 2 * j + 2, mo],
                        rhs8[:, 2 * j : 2 * j + 2, :],
                        start=False, stop=(j == KH // 2 - 1),
                        perf_mode=DR,
                    )

            for blk in range(NBLK):
                sl = bass.ts(blk, BLK)
                if blk + 1 < NBLK:
                    fetch_block(blk + 1)
                xbt, hbt, h8t = xb[blk], hb[blk], h8[blk]

                # ---- R phase: r = sigmoid((x@Wxr + h@Whr)/1 + bxr); rh8 = fp8(r*h)
                # Emit all x-side groups first: on block 0 they only need
                # xT+wxr, so the PE starts before h/Whr even land.
                rh8 = rhpool.tile([P, KH, BLK], FP8, tag="rh8")
                ps_r = []
                for m in range(M):
                    ps = psum.tile([P, BLK], F32, tag="ps", name="ps")
                    ps_r.append(ps)
                    mo = bass.ts(m, P)
                    for k in range(KX):
                        nc.tensor.matmul(
                            ps[:], wxr_s[:, k, mo], xbt[:, k, :],
                            start=(k == 0), stop=False,
                        )
                for m in range(M):
                    ps = ps_r[m]
                    mo = bass.ts(m, P)
                    for j in range(KH // 2):
                        nc.tensor.matmul(
                            ps[:], whr_s[:, 2 * j : 2 * j + 2, mo],
                            h8t[:, 2 * j : 2 * j + 2, :],
                            start=False, stop=(j == KH // 2 - 1),
                            perf_mode=DR,
                        )
                for m in range(M):
                    rt = rpool.tile([P, BLK], F32, tag="rt")
                    nc.scalar.activation(
                        rt[:], ps_r[m][:], SIG,
                        bias=b_s[:, m : m + 1], scale=inv_s,
                    )
                    eng = nc.vector if m % 2 == 0 else nc.gpsimd
                    eng.tensor_mul(rh8[:, m, :], rt[:], hbt[:, m, :])

                # ---- Z phase: z = sigmoid(x@Wxz + bxz + h@Whz)
                zf = zpool.tile([P, M, BLK], F32, tag="zf")
                for m in range(M):
                    ps = psum.tile([P, BLK], F32, tag="ps")
                    mm_group(ps, wxz_s, whz_s, m, xbt, h8t, True)
                    nc.scalar.activation(
                        zf[:, m, :], ps[:], SIG,
                        bias=b_s[:, 8 + m : 9 + m], scale=inv_s,
                    )

                # ---- HC phase: hc = tanh(x@Wxh + bxh + rh@Whh)
                #      out = hc + z*(h - hc)
                for m in range(M):
                    ps = psum.tile([P, BLK], F32, tag="ps")
                    mm_group(ps, wxh_s, whh_s, m, xbt, rh8, True)
                    hct = hcpool.tile([P, BLK], F32, tag="hct")
                    nc.scalar.activation(
                        hct[:], ps[:], TANH,
                        bias=b_s[:, 16 + m : 17 + m], scale=inv_s,
                    )
                    eng = nc.vector if m % 2 == 0 else nc.gpsimd
                    ot = opool.tile([P, BLK], F32, tag="ot")
                    ob = opool.tile([P, BLK], BF16, tag="ob")
                    eng.tensor_sub(ot[:], hbt[:, m, :], hct[:])
                    eng.tensor_mul(ot[:], ot[:], zf[:, m, :])
                    eng.tensor_add(ob[:], ot[:], hct[:])
                    nc.sync.dma_start(outT[:, m, sl], ob[:])

    nc.compile()
    return nc


def _pack_feature_major(a: np.ndarray, nchunks: int, dtype) -> np.ndarray:
    # [rows, cols] -> [128, nchunks, cols] with [p, k, c] = a[128k+p, c]
    rows, cols = a.shape
    assert rows == nchunks * P
    return np.ascontiguousarray(
        a.reshape(nchunks, P, cols).transpose(1, 0, 2)
    ).astype(dtype)


def _pack_inputs(x, hidden, Wxr, bxr, Whr, Wxz, bxz, Whz, Wxh, bxh, Whh):
    bf = ml_dtypes.bfloat16
    f8 = ml_dtypes.float8_e4m3  # TRN-compatible e4m3 (max 240)
    wxr_p = _pack_feature_major(np.asarray(Wxr, np.float32) * WSCALE, KX, bf)
    wxz_p = _pack_feature_major(np.asarray(Wxz, np.float32) * WSCALE, KX, bf)
    wxh_p = _pack_feature_major(np.asarray(Wxh, np.float32) * WSCALE, KX, bf)
    whr_p = _pack_feature_major(np.asarray(Whr, np.float32) * WSCALE, KH, f8)
    whz_p = _pack_feature_major(np.asarray(Whz, np.float32) * WSCALE, KH, f8)
    whh_p = _pack_feature_major(np.asarray(Whh, np.float32) * WSCALE, KH, f8)
    bias_p = np.ascontiguousarray(
        np.concatenate(
            [
                np.asarray(b, np.float32).reshape(M, P).T
                for b in (bxr, bxz, bxh)
            ],
            axis=1,
        )
    )  # [128, 24]

    x = np.asarray(x, np.float32)
    hidden = np.asarray(hidden, np.float32)

    in_maps = []
    for c in range(NCORES):
        rows = slice(c * NB, (c + 1) * NB)
        in_maps.append(
            {
                "xT": _pack_feature_major(x[rows].T, KX, bf),
                "hT": _pack_feature_major(hidden[rows].T, KH, bf),
                "wxr": wxr_p,
                "wxz": wxz_p,
                "wxh": wxh_p,
                "whr": whr_p,
                "whz": whz_p,
                "whh": whh_p,
                "bias": bias_p,
            }
        )
    return in_maps


def kernel(x, hidden, Wxr, bxr, Whr, Wxz, bxz, Whz, Wxh, bxh, Whh):
    if "nc" not in _CACHE:
        _CACHE["nc"] = _build()
    nc = _CACHE["nc"]

    in_maps = _pack_inputs(
        x, hidden, Wxr, bxr, Whr, Wxz, bxz, Whz, Wxh, bxh, Whh
    )
    res = run_bass_kernel_spmd(nc, in_maps, core_ids=list(range(NCORES)))

    out = np.empty((BATCH, HID), np.float32)
    for c in range(NCORES):
        oT = np.asarray(res.results[c]["outT"], dtype=np.float32)  # [128, 8, 2048]
        out[c * NB : (c + 1) * NB] = oT.transpose(1, 0, 2).reshape(HID, NB).T
    return out


# revision 4
# speedup vs baseline: 1.5734x; 1.1610x over previous
"""GRU cell (single timestep) on 8 TRN2 NeuronCores, data-parallel over batch.

Contract: kernel(**inputs) takes FULL numpy inputs (as produced by the
problem's setup_inputs()) and returns the FULL (16384, 1024) float32 output.

Strategy:
  - Shard batch (16384) across 8 cores -> 2048 rows/core. Replicate weights.
  - Host-side packing puts every tensor in feature-major layout so the
    TensorEngine contraction dim is the partition dim:
      xT   [128, 4, 2048]  bf16   [p, k, b] = x[b, 128k+p]
      xT8  [128, 4, 2048]  fp8e4  same values, e4m3
      hT   [128, 8, 2048]  bf16   [p, k, b] = hidden[b, 128k+p]
      hT8  [128, 8, 2048]  fp8e4  same values, e4m3
      Wx*  bf16/fp8        [p, k, o] = 1024*W[128k+p, o]
      Wh*  [128, 8, 1024]  fp8e4  [p, k, o] = e4m3(1024*W[128k+p, o])
      bias [128, 24]       f32    [p, 8g+m] = b_g[128m+p]
      outT [128, 8, 2048]  bf16   [p, m, b] = out[b, 128m+p]
  - All h-side matmuls, and the r-gate x-side, run in fp8 e4m3 with
    perf_mode=DoubleRow (2 k-tiles contracted per MM -> ~1.9x PE rate);
    the z/hc x-side matmuls stay bf16 (fp8 there pushes rel err to ~2e-2).
    All weights are pre-scaled by 1024 (exact pow2 shift for bf16; lifts
    fp8 weights out of the subnormal range); the activation applies
    scale=1/1024 before bias+nonlinearity, so numerics are unchanged.
  - Activations + elementwise combine all in bf16 so tensor_tensor ops hit
    the DVE 2x packed mode; fp32 PSUM accumulation throughout.
  - Late weights and blocks 1-3 are fetched via dma_start on the Scalar
    engine queue, which defers their descriptors until block 0 is in
    flight -- the first matmul starts ~5us in instead of ~35us.
"""

import sys

if "/opt/trn_rl_repo" not in sys.path:
    sys.path.insert(0, "/opt/trn_rl_repo")

import numpy as np
import ml_dtypes

import concourse.bass as bass
import concourse.tile as tile
from concourse import bacc, mybir
from concourse.bass_utils import run_bass_kernel_spmd

P = 128
NCORES = 8
BATCH = 16384
NB = BATCH // NCORES          # 2048 rows per core
IN = 512
HID = 1024
KX = IN // P                  # 4
KH = HID // P                 # 8
M = HID // P                  # 8 output-feature chunks
BLK = 512                     # batch columns per block
NBLK = NB // BLK              # 4
WSCALE = 1024.0               # pow2 pre-scale on all weights

F32 = mybir.dt.float32
BF16 = mybir.dt.bfloat16
FP8 = mybir.dt.float8e4

SIG = mybir.ActivationFunctionType.Sigmoid
TANH = mybir.ActivationFunctionType.Tanh
DR = mybir.MatmulPerfMode.DoubleRow

_CACHE = {}


def _build():
    nc = bacc.Bacc("TRN2", target_bir_lowering=False, debug=False, num_devices=NCORES)

    xT = nc.dram_tensor("xT", [P, KX, NB], BF16, kind="ExternalInput").ap()
    xT8 = nc.dram_tensor("xT8", [P, KX, NB], FP8, kind="ExternalInput").ap()
    hT = nc.dram_tensor("hT", [P, KH, NB], BF16, kind="ExternalInput").ap()
    hT8 = nc.dram_tensor("hT8", [P, KH, NB], FP8, kind="ExternalInput").ap()
    wxr = nc.dram_tensor("wxr", [P, KX, HID], FP8, kind="ExternalInput").ap()
    wxz = nc.dram_tensor("wxz", [P, KX, HID], BF16, kind="ExternalInput").ap()
    wxh = nc.dram_tensor("wxh", [P, KX, HID], BF16, kind="ExternalInput").ap()
    whr = nc.dram_tensor("whr", [P, KH, HID], FP8, kind="ExternalInput").ap()
    whz = nc.dram_tensor("whz", [P, KH, HID], FP8, kind="ExternalInput").ap()
    whh = nc.dram_tensor("whh", [P, KH, HID], FP8, kind="ExternalInput").ap()
    bias = nc.dram_tensor("bias", [P, 24], F32, kind="ExternalInput").ap()
    outT = nc.dram_tensor("outT", [P, M, NB], BF16, kind="ExternalOutput").ap()

    inv_s = 1.0 / WSCALE

    with tile.TileContext(nc) as tc:
        with (
            tc.tile_pool(name="wpool", bufs=1) as wpool,
            tc.tile_pool(name="xpool", bufs=3) as xpool,
            tc.tile_pool(name="x8pool", bufs=3) as x8pool,
            tc.tile_pool(name="hpool", bufs=3) as hpool,
            tc.tile_pool(name="h8pool", bufs=3) as h8pool,
            tc.tile_pool(name="rpool", bufs=2) as rpool,
            tc.tile_pool(name="rhpool", bufs=2) as rhpool,
            tc.tile_pool(name="zpool", bufs=2) as zpool,
            tc.tile_pool(name="hcpool", bufs=2) as hcpool,
            tc.tile_pool(name="opool", bufs=4) as opool,
            tc.tile_pool(name="psum", bufs=8, space=bass.MemorySpace.PSUM) as psum,
        ):
            b_s = wpool.tile([P, 24], F32)
            wxr_s = wpool.tile([P, KX, HID], FP8)
            wxz_s = wpool.tile([P, KX, HID], BF16)
            wxh_s = wpool.tile([P, KX, HID], BF16)
            whr_s = wpool.tile([P, KH, HID], FP8)
            whz_s = wpool.tile([P, KH, HID], FP8)
            whh_s = wpool.tile([P, KH, HID], FP8)

            xb = [None] * NBLK
            x8b = [None] * NBLK
            hb = [None] * NBLK
            h8b = [None] * NBLK

            def fetch_block(blk, eng):
                sl = bass.ts(blk, BLK)
                x8b[blk] = x8pool.tile([P, KX, BLK], FP8, tag="x8b", name="x8b")
                eng.dma_start(x8b[blk][:], xT8[:, :, sl])
                h8b[blk] = h8pool.tile([P, KH, BLK], FP8, tag="h8b", name="h8b")
                eng.dma_start(h8b[blk][:], hT8[:, :, sl])
                hb[blk] = hpool.tile([P, KH, BLK], BF16, tag="hb", name="hb")
                eng.dma_start(hb[blk][:], hT[:, :, sl])
                xb[blk] = xpool.tile([P, KX, BLK], BF16, tag="xb", name="xb")
                eng.dma_start(xb[blk][:], xT[:, :, sl])

            # t=0 critical set on the sync queue, in priority order: the
            # r-gate needs x8+wxr8 first, then h8+whr, then hb (for r*h),
            # then the z-gate weights.
            nc.sync.dma_start(b_s[:], bias[:])
            fetch_block(0, nc.sync)
            nc.sync.dma_start(wxr_s[:], wxr[:])
            nc.sync.dma_start(whr_s[:], whr[:])
            nc.sync.dma_start(wxz_s[:], wxz[:])
            nc.sync.dma_start(whz_s[:], whz[:])

            def x_mms(ps, wx_s, m, xbt, x8t, fp8_x):
                mo = bass.ts(m, P)
                if fp8_x:
                    for j in range(KX // 2):
                        nc.tensor.matmul(
                            ps[:], wx_s[:, 2 * j : 2 * j + 2, mo],
                            x8t[:, 2 * j : 2 * j + 2, :],
                            start=(j == 0), stop=False, perf_mode=DR,
                        )
                else:
                    for k in range(KX):
                        nc.tensor.matmul(
                            ps[:], wx_s[:, k, mo], xbt[:, k, :],
                            start=(k == 0), stop=False,
                        )

            def h_mms(ps, wh_s, m, rhs8):
                mo = bass.ts(m, P)
                for j in range(KH // 2):
                    nc.tensor.matmul(
                        ps[:], wh_s[:, 2 * j : 2 * j + 2, mo],
                        rhs8[:, 2 * j : 2 * j + 2, :],
                        start=False, stop=(j == KH // 2 - 1),
                        perf_mode=DR,
                    )

            for blk in range(NBLK):
                sl = bass.ts(blk, BLK)
                xbt, x8t, hbt, h8t = xb[blk], x8b[blk], hb[blk], h8b[blk]
                split = blk == 0  # emit x-groups before h-groups on block 0

                # ---- R phase: r = sigmoid(x@Wxr + bxr + h@Whr); rh8 = fp8(r*h)
                rh8 = rhpool.tile([P, KH, BLK], FP8, tag="rh8")
                ps_r = []
                for m in range(M):
                    ps = psum.tile([P, BLK], F32, tag="ps", name="ps")
                    ps_r.append(ps)
                    x_mms(ps, wxr_s, m, xbt, x8t, True)
                    if not split:
                        h_mms(ps, whr_s, m, h8t)
                if split:
                    for m in range(M):
                        h_mms(ps_r[m], whr_s, m, h8t)
                for m in range(M):
                    rt = rpool.tile([P, BLK], BF16, tag="rt")
                    nc.scalar.activation(
                        rt[:], ps_r[m][:], SIG,
                        bias=b_s[:, m : m + 1], scale=inv_s,
                    )
                    eng = nc.vector if m % 2 == 0 else nc.gpsimd
                    eng.tensor_mul(rh8[:, m, :], rt[:], hbt[:, m, :])

                if blk == 0:
                    # late weights + next block, deferred to the scalar queue
                    nc.scalar.dma_start(wxh_s[:], wxh[:])
                    nc.scalar.dma_start(whh_s[:], whh[:])
                    fetch_block(1, nc.scalar)

                # ---- Z phase: z = sigmoid(x@Wxz + bxz + h@Whz)
                zf = zpool.tile([P, M, BLK], BF16, tag="zf")
                ps_z = []
                for m in range(M):
                    ps = psum.tile([P, BLK], F32, tag="ps", name="ps")
                    ps_z.append(ps)
                    x_mms(ps, wxz_s, m, xbt, x8t, False)
                    if not split:
                        h_mms(ps, whz_s, m, h8t)
                if split:
                    for m in range(M):
                        h_mms(ps_z[m], whz_s, m, h8t)
                for m in range(M):
                    nc.scalar.activation(
                        zf[:, m, :], ps_z[m][:], SIG,
                        bias=b_s[:, 8 + m : 9 + m], scale=inv_s,
                    )

                if blk + 2 < NBLK:
                    fetch_block(blk + 2, nc.scalar)

                # ---- HC phase: hc = tanh(x@Wxh + bxh + rh@Whh)
                #      out = hc + z*(h - hc)
                for m in range(M):
                    ps = psum.tile([P, BLK], F32, tag="ps", name="ps")
                    x_mms(ps, wxh_s, m, xbt, x8t, False)
                    h_mms(ps, whh_s, m, rh8)
                    hct = hcpool.tile([P, BLK], BF16, tag="hct")
                    nc.scalar.activation(
                        hct[:], ps[:], TANH,
                        bias=b_s[:, 16 + m : 17 + m], scale=inv_s,
                    )
                    ot = opool.tile([P, BLK], BF16, tag="ot")
                    ob = opool.tile([P, BLK], BF16, tag="ob")
                    nc.vector.tensor_sub(ot[:], hbt[:, m, :], hct[:])
                    nc.vector.tensor_mul(ot[:], ot[:], zf[:, m, :])
                    nc.vector.tensor_add(ob[:], ot[:], hct[:])
                    nc.sync.dma_start(outT[:, m, sl], ob[:])

    nc.compile()
    return nc


def _pack_feature_major(a: np.ndarray, nchunks: int, dtype) -> np.ndarray:
    # [rows, cols] -> [128, nchunks, cols] with [p, k, c] = a[128k+p, c]
    rows, cols = a.shape
    assert rows == nchunks * P
    return np.ascontiguousarray(
        a.reshape(nchunks, P, cols).transpose(1, 0, 2)
    ).astype(dtype)


def _pack_inputs(x, hidden, Wxr, bxr, Whr, Wxz, bxz, Whz, Wxh, bxh, Whh):
    bf = ml_dtypes.bfloat16
    f8 = ml_dtypes.float8_e4m3  # TRN-compatible e4m3 (max 240)
    wxr_p = _pack_feature_major(np.asarray(Wxr, np.float32) * WSCALE, KX, f8)
    wxz_p = _pack_feature_major(np.asarray(Wxz, np.float32) * WSCALE, KX, bf)
    wxh_p = _pack_feature_major(np.asarray(Wxh, np.float32) * WSCALE, KX, bf)
    whr_p = _pack_feature_major(np.asarray(Whr, np.float32) * WSCALE, KH, f8)
    whz_p = _pack_feature_major(np.asarray(Whz, np.float32) * WSCALE, KH, f8)
    whh_p = _pack_feature_major(np.asarray(Whh, np.float32) * WSCALE, KH, f8)
    bias_p = np.ascontiguousarray(
        np.concatenate(
            [
                np.asarray(b, np.float32).reshape(M, P).T
                for b in (bxr, bxz, bxh)
            ],
            axis=1,
        )
    )  # [128, 24]

    x = np.asarray(x, np.float32)
    hidden = np.asarray(hidden, np.float32)

    in_maps = []
    for c in range(NCORES):
        rows = slice(c * NB, (c + 1) * NB)
        xTc = x[rows].T
        hTc = hidden[rows].T
        in_maps.append(
            {
                "xT": _pack_feature_major(xTc, KX, bf),
                "xT8": _pack_feature_major(xTc, KX, f8),
                "hT": _pack_feature_major(hTc, KH, bf),
                "hT8": _pack_feature_major(hTc, KH, f8),
                "wxr": wxr_p,
                "wxz": wxz_p,
                "wxh": wxh_p,
                "whr": whr_p,
                "whz": whz_p,
                "whh": whh_p,
                "bias": bias_p,
            }
        )
    return in_maps


def kernel(x, hidden, Wxr, bxr, Whr, Wxz, bxz, Whz, Wxh, bxh, Whh):
    if "nc" not in _CACHE:
        _CACHE["nc"] = _build()
    nc = _CACHE["nc"]

    in_maps = _pack_inputs(
        x, hidden, Wxr, bxr, Whr, Wxz, bxz, Whz, Wxh, bxh, Whh
    )
    res = run_bass_kernel_spmd(nc, in_maps, core_ids=list(range(NCORES)))

    out = np.empty((BATCH, HID), np.float32)
    for c in range(NCORES):
        oT = np.asarray(res.results[c]["outT"], dtype=np.float32)  # [128, 8, 2048]
        out[c * NB : (c + 1) * NB] = oT.transpose(1, 0, 2).reshape(HID, NB).T
    return out


# revision 6
# speedup vs baseline: 1.5758x; 1.0015x over previous
"""GRU cell (single timestep) on 8 TRN2 NeuronCores, data-parallel over batch.

Contract: kernel(**inputs) takes FULL numpy inputs (as produced by the
problem's setup_inputs()) and returns the FULL (16384, 1024) float32 output.

Strategy:
  - Shard batch (16384) across 8 cores -> 2048 rows/core; 4 column-blocks
    of 512 per core. Replicate weights.
  - Block-major DRAM layouts: every per-block DMA is 128 descriptors of
    2-12KB (contiguous per partition). Strided 512B/1KB-descriptor DMAs
    cost ~4us of descriptor generation on the issuing sequencer and were
    the head bottleneck in earlier versions.
      xh8  [4, 128, 12, 512] fp8   k 0-3 = x/e4m3, k 4-11 = hidden/e4m3
      xhb  [4, 128, 12, 512] bf16  k 0-3 = x,      k 4-11 = hidden
      Wx*  [128, 4, 1024]  fp8(r)/bf16(z,hc)   [p,k,o] = 1024*W[128k+p,o]
      Wh*  [128, 8, 1024]  fp8                 [p,k,o] = e4m3(1024*W[...])
      bias [128, 24] f32;  outT [4, 128, 8, 512] bf16
  - All h-side matmuls, and the r-gate x-side, run in fp8 e4m3 with
    perf_mode=DoubleRow (2 k-tiles per MM -> ~1.9x PE rate); z/hc x-side
    stays bf16 (fp8 there pushes rel err too close to the gate).
    All weights pre-scaled by 1024 (exact pow2 shift for bf16; lifts fp8
    weights out of the subnormal range); activation applies scale=1/1024.
  - Activations + elementwise combine in bf16 (DVE 2x packed mode);
    fp32 PSUM accumulation.
  - Late weights and blocks 1-3 fetched via dma_start on the Scalar
    queue so block-0 transfers get the full HBM bandwidth first.
"""

import sys

if "/opt/trn_rl_repo" not in sys.path:
    sys.path.insert(0, "/opt/trn_rl_repo")

import numpy as np
import ml_dtypes

import concourse.bass as bass
import concourse.tile as tile
from concourse import bacc, mybir
from concourse.bass_utils import run_bass_kernel_spmd

P = 128
NCORES = 8
BATCH = 16384
NB = BATCH // NCORES          # 2048 rows per core
IN = 512
HID = 1024
KX = IN // P                  # 4
KH = HID // P                 # 8
KA = KX + KH                  # 12 packed k-chunks (x then h)
M = HID // P                  # 8 output-feature chunks
BLK = 512                     # batch columns per block
NBLK = NB // BLK              # 4
WSCALE = 1024.0               # pow2 pre-scale on all weights

F32 = mybir.dt.float32
BF16 = mybir.dt.bfloat16
FP8 = mybir.dt.float8e4

SIG = mybir.ActivationFunctionType.Sigmoid
TANH = mybir.ActivationFunctionType.Tanh
DR = mybir.MatmulPerfMode.DoubleRow

_CACHE = {}


def _build():
    nc = bacc.Bacc("TRN2", target_bir_lowering=False, debug=False, num_devices=NCORES)

    xh8 = nc.dram_tensor("xh8", [NBLK, P, KA, BLK], FP8, kind="ExternalInput").ap()
    xhb = nc.dram_tensor("xhb", [NBLK, P, KA, BLK], BF16, kind="ExternalInput").ap()
    wxr = nc.dram_tensor("wxr", [P, KX, HID], FP8, kind="ExternalInput").ap()
    wxz = nc.dram_tensor("wxz", [P, KX, HID], BF16, kind="ExternalInput").ap()
    wxh = nc.dram_tensor("wxh", [P, KX, HID], BF16, kind="ExternalInput").ap()
    whr = nc.dram_tensor("whr", [P, KH, HID], FP8, kind="ExternalInput").ap()
    whz = nc.dram_tensor("whz", [P, KH, HID], FP8, kind="ExternalInput").ap()
    whh = nc.dram_tensor("whh", [P, KH, HID], FP8, kind="ExternalInput").ap()
    bias = nc.dram_tensor("bias", [P, 24], F32, kind="ExternalInput").ap()
    outT = nc.dram_tensor("outT", [NBLK, P, M, BLK], BF16, kind="ExternalOutput").ap()

    inv_s = 1.0 / WSCALE

    with tile.TileContext(nc) as tc:
        with (
            tc.tile_pool(name="wpool", bufs=1) as wpool,
            tc.tile_pool(name="a8pool", bufs=3) as a8pool,
            tc.tile_pool(name="abpool", bufs=3) as abpool,
            tc.tile_pool(name="rpool", bufs=2) as rpool,
            tc.tile_pool(name="rhpool", bufs=2) as rhpool,
            tc.tile_pool(name="zpool", bufs=2) as zpool,
            tc.tile_pool(name="hcpool", bufs=2) as hcpool,
            tc.tile_pool(name="opool", bufs=4) as opool,
            tc.tile_pool(name="obpool", bufs=2) as obpool,
            tc.tile_pool(name="psum", bufs=8, space=bass.MemorySpace.PSUM) as psum,
        ):
            b_s = wpool.tile([P, 24], F32)
            wxr_s = wpool.tile([P, KX, HID], FP8)
            wxz_s = wpool.tile([P, KX, HID], BF16)
            wxh_s = wpool.tile([P, KX, HID], BF16)
            whr_s = wpool.tile([P, KH, HID], FP8)
            whz_s = wpool.tile([P, KH, HID], FP8)
            whh_s = wpool.tile([P, KH, HID], FP8)

            a8 = [None] * NBLK   # [P, 12, BLK] fp8: x8 k0-3, h8 k4-11
            ab = [None] * NBLK   # [P, 12, BLK] bf16: xb k0-3, hb k4-11

            def fetch_block(blk, eng):
                a8[blk] = a8pool.tile([P, KA, BLK], FP8, tag="a8", name="a8")
                eng.dma_start(a8[blk][:], xh8[blk])
                ab[blk] = abpool.tile([P, KA, BLK], BF16, tag="ab", name="ab")
                eng.dma_start(ab[blk][:], xhb[blk])

            # t=0 critical set on the sync queue, in priority order: the
            # r-gate needs x8(+wxr8) and h8(+whr) first, then bias for the
            # activations, then the z weights, then bf16 x/h.
            a8[0] = a8pool.tile([P, KA, BLK], FP8, tag="a8", name="a8")
            nc.sync.dma_start(a8[0][:], xh8[0])
            nc.sync.dma_start(wxr_s[:], wxr[:])
            nc.sync.dma_start(whr_s[:], whr[:])
            nc.sync.dma_start(b_s[:], bias[:])
            ab[0] = abpool.tile([P, KA, BLK], BF16, tag="ab", name="ab")
            nc.sync.dma_start(ab[0][:], xhb[0])
            nc.sync.dma_start(wxz_s[:], wxz[:])
            nc.sync.dma_start(whz_s[:], whz[:])

            def x_mms(ps, wx_s, m, a8t, abt, fp8_x):
                mo = bass.ts(m, P)
                if fp8_x:
                    for j in range(KX // 2):
                        nc.tensor.matmul(
                            ps[:], wx_s[:, 2 * j : 2 * j + 2, mo],
                            a8t[:, 2 * j : 2 * j + 2, :],
                            start=(j == 0), stop=False, perf_mode=DR,
                        )
                else:
                    for k in range(KX):
                        nc.tensor.matmul(
                            ps[:], wx_s[:, k, mo], abt[:, k, :],
                            start=(k == 0), stop=False,
                        )

            def h_mms(ps, wh_s, m, rhs8, koff):
                # rhs8 k-chunks [koff, koff+8) hold the 8 hidden chunks
                mo = bass.ts(m, P)
                for j in range(KH // 2):
                    nc.tensor.matmul(
                        ps[:], wh_s[:, 2 * j : 2 * j + 2, mo],
                        rhs8[:, koff + 2 * j : koff + 2 * j + 2, :],
                        start=False, stop=(j == KH // 2 - 1),
                        perf_mode=DR,
                    )

            for blk in range(NBLK):
                a8t, abt = a8[blk], ab[blk]
                split = blk == 0  # emit x-groups before h-groups on block 0

                # ---- R phase: r = sigmoid(x@Wxr + bxr + h@Whr); rh8 = fp8(r*h)
                rh8 = rhpool.tile([P, KH, BLK], FP8, tag="rh8")
                ps_r = []
                for m in range(M):
                    ps = psum.tile([P, BLK], F32, tag="ps", name="ps")
                    ps_r.append(ps)
                    x_mms(ps, wxr_s, m, a8t, abt, True)
                    if not split:
                        h_mms(ps, whr_s, m, a8t, KX)
                if split:
                    for m in range(M):
                        h_mms(ps_r[m], whr_s, m, a8t, KX)
                for m in range(M):
                    rt = rpool.tile([P, BLK], BF16, tag="rt")
                    nc.scalar.activation(
                        rt[:], ps_r[m][:], SIG,
                        bias=b_s[:, m : m + 1], scale=inv_s,
                    )
                    eng = nc.vector if m % 2 == 0 else nc.gpsimd
                    eng.tensor_mul(rh8[:, m, :], rt[:], abt[:, KX + m, :])

                if blk == 0:
                    # late weights + next block, deferred to the scalar queue
                    nc.scalar.dma_start(wxh_s[:], wxh[:])
                    nc.scalar.dma_start(whh_s[:], whh[:])
                    fetch_block(1, nc.scalar)

                # ---- Z phase: z = sigmoid(x@Wxz + bxz + h@Whz)
                zf = zpool.tile([P, M, BLK], BF16, tag="zf")
                ps_z = []
                for m in range(M):
                    ps = psum.tile([P, BLK], F32, tag="ps", name="ps")
                    ps_z.append(ps)
                    x_mms(ps, wxz_s, m, a8t, abt, False)
                    if not split:
                        h_mms(ps, whz_s, m, a8t, KX)
                if split:
                    for m in range(M):
                        h_mms(ps_z[m], whz_s, m, a8t, KX)
                for m in range(M):
                    nc.scalar.activation(
                        zf[:, m, :], ps_z[m][:], SIG,
                        bias=b_s[:, 8 + m : 9 + m], scale=inv_s,
                    )

                if blk + 2 < NBLK:
                    fetch_block(blk + 2, nc.scalar)

                # ---- HC phase: hc = tanh(x@Wxh + bxh + rh@Whh)
                #      out = hc + z*(h - hc)
                ob = obpool.tile([P, M, BLK], BF16, tag="ob")
                for m in range(M):
                    ps = psum.tile([P, BLK], F32, tag="ps", name="ps")
                    x_mms(ps, wxh_s, m, a8t, abt, False)
                    h_mms(ps, whh_s, m, rh8, 0)
                    hct = hcpool.tile([P, BLK], BF16, tag="hct")
                    nc.scalar.activation(
                        hct[:], ps[:], TANH,
                        bias=b_s[:, 16 + m : 17 + m], scale=inv_s,
                    )
                    ot = opool.tile([P, BLK], BF16, tag="ot")
                    nc.vector.tensor_sub(ot[:], abt[:, KX + m, :], hct[:])
                    nc.vector.tensor_mul(ot[:], ot[:], zf[:, m, :])
                    nc.vector.tensor_add(ob[:, m, :], ot[:], hct[:])
                    if m % 2 == 1:
                        nc.sync.dma_start(
                            outT[blk, :, m - 1 : m + 1, :], ob[:, m - 1 : m + 1, :]
                        )

    nc.compile()
    return nc


def _pack_feature_major(a: np.ndarray, nchunks: int, dtype) -> np.ndarray:
    # [rows, cols] -> [128, nchunks, cols] with [p, k, c] = a[128k+p, c]
    rows, cols = a.shape
    assert rows == nchunks * P
    return np.ascontiguousarray(
        a.reshape(nchunks, P, cols).transpose(1, 0, 2)
    ).astype(dtype)


def _block_major(a: np.ndarray) -> np.ndarray:
    # [128, K, NB] -> [NBLK, 128, K, BLK]
    p, k, nb = a.shape
    return np.ascontiguousarray(
        a.reshape(p, k, NBLK, BLK).transpose(2, 0, 1, 3)
    )


def _pack_inputs(x, hidden, Wxr, bxr, Whr, Wxz, bxz, Whz, Wxh, bxh, Whh):
    bf = ml_dtypes.bfloat16
    f8 = ml_dtypes.float8_e4m3  # TRN-compatible e4m3 (max 240)
    wxr_p = _pack_feature_major(np.asarray(Wxr, np.float32) * WSCALE, KX, f8)
    wxz_p = _pack_feature_major(np.asarray(Wxz, np.float32) * WSCALE, KX, bf)
    wxh_p = _pack_feature_major(np.asarray(Wxh, np.float32) * WSCALE, KX, bf)
    whr_p = _pack_feature_major(np.asarray(Whr, np.float32) * WSCALE, KH, f8)
    whz_p = _pack_feature_major(np.asarray(Whz, np.float32) * WSCALE, KH, f8)
    whh_p = _pack_feature_major(np.asarray(Whh, np.float32) * WSCALE, KH, f8)
    bias_p = np.ascontiguousarray(
        np.concatenate(
            [
                np.asarray(b, np.float32).reshape(M, P).T
                for b in (bxr, bxz, bxh)
            ],
            axis=1,
        )
    )  # [128, 24]

    x = np.asarray(x, np.float32)
    hidden = np.asarray(hidden, np.float32)

    in_maps = []
    for c in range(NCORES):
        rows = slice(c * NB, (c + 1) * NB)
        xTc = x[rows].T
        hTc = hidden[rows].T
        xh = np.concatenate(
            [_pack_feature_major(xTc, KX, np.float32),
             _pack_feature_major(hTc, KH, np.float32)],
            axis=1,
        )  # [128, 12, 2048] f32
        in_maps.append(
            {
                "xh8": _block_major(xh.astype(f8)),
                "xhb": _block_major(xh.astype(bf)),
                "wxr": wxr_p,
                "wxz": wxz_p,
                "wxh": wxh_p,
                "whr": whr_p,
                "whz": whz_p,
                "whh": whh_p,
                "bias": bias_p,
            }
        )
    return in_maps


def kernel(x, hidden, Wxr, bxr, Whr, Wxz, bxz, Whz, Wxh, bxh, Whh):
    if "nc" not in _CACHE:
        _CACHE["nc"] = _build()
    nc = _CACHE["nc"]

    in_maps = _pack_inputs(
        x, hidden, Wxr, bxr, Whr, Wxz, bxz, Whz, Wxh, bxh, Whh
    )
    res = run_bass_kernel_spmd(nc, in_maps, core_ids=list(range(NCORES)))

    out = np.empty((BATCH, HID), np.float32)
    for c in range(NCORES):
        oT = np.asarray(res.results[c]["outT"], dtype=np.float32)  # [4,128,8,512]
        out[c * NB : (c + 1) * NB] = (
            oT.transpose(2, 1, 0, 3).reshape(HID, NB).T
        )
    return out


# revision 7
# speedup vs baseline: 1.6867x; 1.0704x over previous
"""GRU cell (single timestep) on 8 TRN2 NeuronCores, data-parallel over batch.

Contract: kernel(**inputs) takes FULL numpy inputs (as produced by the
problem's setup_inputs()) and returns the FULL (16384, 1024) float32 output.

Strategy:
  - Shard batch (16384) across 8 cores -> 2048 rows/core; 4 column-blocks
    of 512 per core. Replicate weights.
  - Block-major DRAM layouts: every per-block DMA is 128 descriptors of
    2-8KB (contiguous per partition). Strided 512B/1KB-descriptor DMAs
    cost ~4us of descriptor generation on the issuing sequencer.
      xh8  [4, 128, 12, 512] fp8   k 0-3 = x/e4m3, k 4-11 = hidden/e4m3
      xhb  [4, 128, 12, 512] bf16  k 0-3 = x,      k 4-11 = hidden
      Wx*  [128, 4, 1024]  fp8(r,z)/bf16(hc)  [p,k,o] = 1024*W[128k+p,o]
      Wh*  [128, 8, 1024]  fp8                [p,k,o] = e4m3(1024*W[...])
      bias [128, 24] f32;  outT [4, 128, 8, 512] bf16
  - All h-side matmuls, and the r/z-gate x-side, run in fp8 e4m3 with
    perf_mode=DoubleRow (2 k-tiles per MM -> ~1.9x PE rate); the hc-gate
    x-side stays bf16 (fp8 there pushes rel err to ~2e-2; this config
    measures ~1.5e-2 vs the 2e-2 gate). All weights pre-scaled by 1024
    (exact pow2 shift for bf16; lifts fp8 weights out of the subnormal
    range); the activation applies scale=1/1024 before bias+nonlinearity.
  - Activations + elementwise combine in bf16 (DVE 2x packed mode);
    fp32 PSUM accumulation.
  - Head optimizations: DMA rings serve in-flight transfers round-robin,
    so only the r-gate working set is issued at t=0; the z weights are
    released by a tiny GpSimd fence op gated on the block-0 fp8 arrival,
    and hc weights + blocks 1-3 issue from the Scalar queue after the
    first activations. A dozen dummy matmuls on a memset tile warm the
    PE clock (HAM 1.2->2.4GHz) while the first DMAs land.
"""

import sys

if "/opt/trn_rl_repo" not in sys.path:
    sys.path.insert(0, "/opt/trn_rl_repo")

import numpy as np
import ml_dtypes

import concourse.bass as bass
import concourse.tile as tile
from concourse import bacc, mybir
from concourse.bass_utils import run_bass_kernel_spmd

P = 128
NCORES = 8
BATCH = 16384
NB = BATCH // NCORES          # 2048 rows per core
IN = 512
HID = 1024
KX = IN // P                  # 4
KH = HID // P                 # 8
KA = KX + KH                  # 12 packed k-chunks (x then h)
M = HID // P                  # 8 output-feature chunks
BLK = 512                     # batch columns per block
NBLK = NB // BLK              # 4
WSCALE = 1024.0               # pow2 pre-scale on all weights
Z_X_FP8 = True                # z-gate x-side in fp8 DoubleRow
N_WARM = 12                   # dummy matmuls to warm the PE clock

F32 = mybir.dt.float32
BF16 = mybir.dt.bfloat16
FP8 = mybir.dt.float8e4

SIG = mybir.ActivationFunctionType.Sigmoid
TANH = mybir.ActivationFunctionType.Tanh
DR = mybir.MatmulPerfMode.DoubleRow

_CACHE = {}


def _build():
    nc = bacc.Bacc("TRN2", target_bir_lowering=False, debug=False, num_devices=NCORES)

    xh8 = nc.dram_tensor("xh8", [NBLK, P, KA, BLK], FP8, kind="ExternalInput").ap()
    xhb = nc.dram_tensor("xhb", [NBLK, P, KA, BLK], BF16, kind="ExternalInput").ap()
    wxr = nc.dram_tensor("wxr", [P, KX, HID], FP8, kind="ExternalInput").ap()
    wxz = nc.dram_tensor(
        "wxz", [P, KX, HID], FP8 if Z_X_FP8 else BF16, kind="ExternalInput"
    ).ap()
    wxh = nc.dram_tensor("wxh", [P, KX, HID], BF16, kind="ExternalInput").ap()
    whr = nc.dram_tensor("whr", [P, KH, HID], FP8, kind="ExternalInput").ap()
    whz = nc.dram_tensor("whz", [P, KH, HID], FP8, kind="ExternalInput").ap()
    whh = nc.dram_tensor("whh", [P, KH, HID], FP8, kind="ExternalInput").ap()
    bias = nc.dram_tensor("bias", [P, 24], F32, kind="ExternalInput").ap()
    outT = nc.dram_tensor("outT", [NBLK, P, M, BLK], BF16, kind="ExternalOutput").ap()

    inv_s = 1.0 / WSCALE

    with tile.TileContext(nc) as tc:
        with (
            tc.tile_pool(name="wpool", bufs=1) as wpool,
            tc.tile_pool(name="x8pool", bufs=3) as x8pool,
            tc.tile_pool(name="h8pool", bufs=3) as h8pool,
            tc.tile_pool(name="xbpool", bufs=3) as xbpool,
            tc.tile_pool(name="hbpool", bufs=3) as hbpool,
            tc.tile_pool(name="rpool", bufs=2) as rpool,
            tc.tile_pool(name="rhpool", bufs=2) as rhpool,
            tc.tile_pool(name="zpool", bufs=2) as zpool,
            tc.tile_pool(name="hcpool", bufs=2) as hcpool,
            tc.tile_pool(name="opool", bufs=4) as opool,
            tc.tile_pool(name="obpool", bufs=2) as obpool,
            tc.tile_pool(name="psum", bufs=8, space=bass.MemorySpace.PSUM) as psum,
        ):
            b_s = wpool.tile([P, 24], F32)
            wxr_s = wpool.tile([P, KX, HID], FP8)
            wxz_s = wpool.tile([P, KX, HID], FP8 if Z_X_FP8 else BF16)
            wxh_s = wpool.tile([P, KX, HID], BF16)
            whr_s = wpool.tile([P, KH, HID], FP8)
            whz_s = wpool.tile([P, KH, HID], FP8)
            whh_s = wpool.tile([P, KH, HID], FP8)
            dummy = wpool.tile([P, BLK], BF16)
            fence = wpool.tile([P, 2], FP8)

            x8 = [None] * NBLK   # [P, 4, BLK] fp8
            h8 = [None] * NBLK   # [P, 8, BLK] fp8
            xb = [None] * NBLK   # [P, 4, BLK] bf16
            hb = [None] * NBLK   # [P, 8, BLK] bf16

            def fetch_fp8(blk, eng):
                x8[blk] = x8pool.tile([P, KX, BLK], FP8, tag="x8", name="x8")
                eng.dma_start(x8[blk][:], xh8[blk, :, 0:KX, :])
                h8[blk] = h8pool.tile([P, KH, BLK], FP8, tag="h8", name="h8")
                eng.dma_start(h8[blk][:], xh8[blk, :, KX:KA, :])

            def fetch_bf(blk, eng):
                hb[blk] = hbpool.tile([P, KH, BLK], BF16, tag="hb", name="hb")
                eng.dma_start(hb[blk][:], xhb[blk, :, KX:KA, :])
                xb[blk] = xbpool.tile([P, KX, BLK], BF16, tag="xb", name="xb")
                eng.dma_start(xb[blk][:], xhb[blk, :, 0:KX, :])

            # PE warmup: dummy matmuls on a memset tile while DMAs land.
            nc.vector.memset(dummy[:], 0.0)
            warm_ps = psum.tile([P, BLK], F32, tag="ps", name="ps")
            for _ in range(N_WARM):
                nc.tensor.matmul(
                    warm_ps[:], dummy[:, 0:P], dummy[:], start=True, stop=True,
                )

            # t=0 critical set (sync queue): the r-gate working set only.
            nc.sync.dma_start(b_s[:], bias[:])
            fetch_fp8(0, nc.sync)
            nc.sync.dma_start(wxr_s[:], wxr[:])
            nc.sync.dma_start(whr_s[:], whr[:])
            fetch_bf(0, nc.sync)

            # z-weight release: gated on block-0 fp8 arrival via a tiny
            # GpSimd copy, so these transfers don't steal bandwidth from
            # the critical set.
            nc.gpsimd.tensor_copy(fence[:], x8[0][:, 0, 0:2])
            nc.gpsimd.dma_start(wxz_s[:], wxz[:])
            nc.gpsimd.dma_start(whz_s[:], whz[:])

            def x_mms(ps, wx_s, m, x8t, xbt, fp8_x):
                mo = bass.ts(m, P)
                if fp8_x:
                    for j in range(KX // 2):
                        nc.tensor.matmul(
                            ps[:], wx_s[:, 2 * j : 2 * j + 2, mo],
                            x8t[:, 2 * j : 2 * j + 2, :],
                            start=(j == 0), stop=False, perf_mode=DR,
                        )
                else:
                    for k in range(KX):
                        nc.tensor.matmul(
                            ps[:], wx_s[:, k, mo], xbt[:, k, :],
                            start=(k == 0), stop=False,
                        )

            def h_mms(ps, wh_s, m, rhs8):
                mo = bass.ts(m, P)
                for j in range(KH // 2):
                    nc.tensor.matmul(
                        ps[:], wh_s[:, 2 * j : 2 * j + 2, mo],
                        rhs8[:, 2 * j : 2 * j + 2, :],
                        start=False, stop=(j == KH // 2 - 1),
                        perf_mode=DR,
                    )

            for blk in range(NBLK):
                x8t, h8t, xbt, hbt = x8[blk], h8[blk], xb[blk], hb[blk]
                split = blk == 0  # emit x-groups before h-groups on block 0

                # ---- R phase: r = sigmoid(x@Wxr + bxr + h@Whr); rh8 = fp8(r*h)
                rh8 = rhpool.tile([P, KH, BLK], FP8, tag="rh8")
                ps_r = []
                for m in range(M):
                    ps = psum.tile([P, BLK], F32, tag="ps", name="ps")
                    ps_r.append(ps)
                    x_mms(ps, wxr_s, m, x8t, xbt, True)
                    if not split:
                        h_mms(ps, whr_s, m, h8t)
                if split:
                    for m in range(M):
                        h_mms(ps_r[m], whr_s, m, h8t)
                for m in range(M):
                    rt = rpool.tile([P, BLK], BF16, tag="rt")
                    nc.scalar.activation(
                        rt[:], ps_r[m][:], SIG,
                        bias=b_s[:, m : m + 1], scale=inv_s,
                    )
                    eng = nc.vector if m % 2 == 0 else nc.gpsimd
                    eng.tensor_mul(rh8[:, m, :], rt[:], hbt[:, m, :])

                if blk == 0:
                    # hc weights + next block, deferred to the scalar queue
                    nc.scalar.dma_start(wxh_s[:], wxh[:])
                    nc.scalar.dma_start(whh_s[:], whh[:])
                    fetch_fp8(1, nc.scalar)
                    fetch_bf(1, nc.scalar)

                # ---- Z phase: z = sigmoid(x@Wxz + bxz + h@Whz)
                zf = zpool.tile([P, M, BLK], BF16, tag="zf")
                ps_z = []
                for m in range(M):
                    ps = psum.tile([P, BLK], F32, tag="ps", name="ps")
                    ps_z.append(ps)
                    x_mms(ps, wxz_s, m, x8t, xbt, Z_X_FP8)
                    if not split:
                        h_mms(ps, whz_s, m, h8t)
                if split:
                    for m in range(M):
                        h_mms(ps_z[m], whz_s, m, h8t)
                for m in range(M):
                    nc.scalar.activation(
                        zf[:, m, :], ps_z[m][:], SIG,
                        bias=b_s[:, 8 + m : 9 + m], scale=inv_s,
                    )

                if blk + 2 < NBLK:
                    fetch_fp8(blk + 2, nc.scalar)
                    fetch_bf(blk + 2, nc.scalar)

                # ---- HC phase: hc = tanh(x@Wxh + bxh + rh@Whh)
                #      out = hc + z*(h - hc)
                ob = obpool.tile([P, M, BLK], BF16, tag="ob")
                for m in range(M):
                    ps = psum.tile([P, BLK], F32, tag="ps", name="ps")
                    x_mms(ps, wxh_s, m, x8t, xbt, False)
                    h_mms(ps, whh_s, m, rh8)
                    hct = hcpool.tile([P, BLK], BF16, tag="hct")
                    nc.scalar.activation(
                        hct[:], ps[:], TANH,
                        bias=b_s[:, 16 + m : 17 + m], scale=inv_s,
                    )
                    ot = opool.tile([P, BLK], BF16, tag="ot")
                    nc.vector.tensor_sub(ot[:], hbt[:, m, :], hct[:])
                    nc.vector.tensor_mul(ot[:], ot[:], zf[:, m, :])
                    nc.vector.tensor_add(ob[:, m, :], ot[:], hct[:])
                    if m % 2 == 1:
                        nc.sync.dma_start(
                            outT[blk, :, m - 1 : m + 1, :], ob[:, m - 1 : m + 1, :]
                        )

    nc.compile()
    return nc


def _pack_feature_major(a: np.ndarray, nchunks: int, dtype) -> np.ndarray:
    # [rows, cols] -> [128, nchunks, cols] with [p, k, c] = a[128k+p, c]
    rows, cols = a.shape
    assert rows == nchunks * P
    return np.ascontiguousarray(
        a.reshape(nchunks, P, cols).transpose(1, 0, 2)
    ).astype(dtype)


def _block_major(a: np.ndarray) -> np.ndarray:
    # [128, K, NB] -> [NBLK, 128, K, BLK]
    p, k, nb = a.shape
    return np.ascontiguousarray(
        a.reshape(p, k, NBLK, BLK).transpose(2, 0, 1, 3)
    )


def _pack_inputs(x, hidden, Wxr, bxr, Whr, Wxz, bxz, Whz, Wxh, bxh, Whh):
    bf = ml_dtypes.bfloat16
    f8 = ml_dtypes.float8_e4m3  # TRN-compatible e4m3 (max 240)
    wxr_p = _pack_feature_major(np.asarray(Wxr, np.float32) * WSCALE, KX, f8)
    wxz_p = _pack_feature_major(
        np.asarray(Wxz, np.float32) * WSCALE, KX, f8 if Z_X_FP8 else bf
    )
    wxh_p = _pack_feature_major(np.asarray(Wxh, np.float32) * WSCALE, KX, bf)
    whr_p = _pack_feature_major(np.asarray(Whr, np.float32) * WSCALE, KH, f8)
    whz_p = _pack_feature_major(np.asarray(Whz, np.float32) * WSCALE, KH, f8)
    whh_p = _pack_feature_major(np.asarray(Whh, np.float32) * WSCALE, KH, f8)
    bias_p = np.ascontiguousarray(
        np.concatenate(
            [
                np.asarray(b, np.float32).reshape(M, P).T
                for b in (bxr, bxz, bxh)
            ],
            axis=1,
        )
    )  # [128, 24]

    x = np.asarray(x, np.float32)
    hidden = np.asarray(hidden, np.float32)

    in_maps = []
    for c in range(NCORES):
        rows = slice(c * NB, (c + 1) * NB)
        xh = np.concatenate(
            [_pack_feature_major(x[rows].T, KX, np.float32),
             _pack_feature_major(hidden[rows].T, KH, np.float32)],
            axis=1,
        )  # [128, 12, 2048] f32
        in_maps.append(
            {
                "xh8": _block_major(xh.astype(f8)),
                "xhb": _block_major(xh.astype(bf)),
                "wxr": wxr_p,
                "wxz": wxz_p,
                "wxh": wxh_p,
                "whr": whr_p,
                "whz": whz_p,
                "whh": whh_p,
                "bias": bias_p,
            }
        )
    return in_maps


def kernel(x, hidden, Wxr, bxr, Whr, Wxz, bxz, Whz, Wxh, bxh, Whh):
    if "nc" not in _CACHE:
        _CACHE["nc"] = _build()
    nc = _CACHE["nc"]

    in_maps = _pack_inputs(
        x, hidden, Wxr, bxr, Whr, Wxz, bxz, Whz, Wxh, bxh, Whh
    )
    res = run_bass_kernel_spmd(nc, in_maps, core_ids=list(range(NCORES)))

    out = np.empty((BATCH, HID), np.float32)
    for c in range(NCORES):
        oT = np.asarray(res.results[c]["outT"], dtype=np.float32)  # [4,128,8,512]
        out[c * NB : (c + 1) * NB] = (
            oT.transpose(2, 1, 0, 3).reshape(HID, NB).T
        )
    return out
